# revision 4
# baseline (speedup 1.0000x reference)
"""Bahdanau additive attention on 8 TRN2 NeuronCores.

Problem shapes: encoder [4, 1024, 256], decoder [4, 256, 256],
W_a/U_a [256, 256], V_a [256, 1].
reference:
    enc_proj = enc @ W_a                  [B, E, H]
    dec_proj = dec @ U_a                  [B, D, H]
    score[b,d,e] = sum_h V[h] * tanh(dec_proj[b,d,h] + enc_proj[b,e,h])
    w = softmax(score, axis=-1)           [B, D, E]
    ctx = w @ enc                         [B, D, H]
    return (ctx, w)

Sharding: 8 cores = (batch b = core//2) x (decoder-row half = core%2).
Each core owns 128 decoder rows of one batch element; outputs are
disjoint so no collectives are needed.

Per-core dataflow (h lives on SBUF partitions, 2 chunks of 128):
  - TensorE: enc_projT[h,e], dec_projT[h,d] projections (contract over
    h_in which sits on partitions; host ships transposed layouts).
  - VectorE: bf16 tensor_scalar pre-add (4x mode) builds
    arg[h, (d,c,e)] = enc_projT[h,e] + dec_projT[h,d] into a wide
    buffer, one instruction per (d, chunk).
  - ScalarE: ONE giant in-place tanh ACTIVATE per group of decoder
    rows (free dim up to 32K) — amortizes the ~224-cycle fixed cost.
    Group sizes ramp [2,4,8,16,...,16,2]: small first group starts the
    tanh stream early, small last group shrinks the PE tail.
  - TensorE: score rows via matmul with a shifted-V trick: lhsT is a
    128-col slice of a [128, 256] tensor whose only nonzero column
    (at index 128) holds V; slice [128-d, 256-d) puts V in column d,
    so PSUM row d accumulates score[d, :] while other rows get += 0.
  - softmax along free axis (exp with accum_out; max-subtraction is
    skipped — |score| <= sum|V| ~ 10 so fp32 exp cannot overflow),
    TensorE transposes of w, bf16 context matmul, DMA out.
"""

import os
import sys

for _p in (
    "/opt/trn_rl_repo",
    "/root/.axon_site",
    "/root/.axon_site/_ro/trn_rl_repo",
    "/root/.axon_site/_ro/pypackages",
):
    if os.path.isdir(_p) and _p not in sys.path:
        sys.path.append(_p)

import ml_dtypes
import numpy as np

import concourse.mybir as mybir
from concourse import bacc, bass, tile

F32 = mybir.dt.float32
BF16 = mybir.dt.bfloat16

B, T_ENC, T_DEC, H = 4, 1024, 256, 256
P = 128  # SBUF partitions
HC = H // P  # h chunks (2)
DPC = 128  # decoder rows per core
N_CORES = 8

# group-size schedule over the 128 decoder rows
GROUPS = [2, 4, 8, 16, 16, 16, 16, 16, 16, 16, 2]
assert sum(GROUPS) == DPC
GMAX = max(GROUPS)

MODE = os.environ.get("ATTN_KERNEL_MODE", "v3")


def build_graph(mode=MODE):
    nc = bacc.Bacc("TRN2", target_bir_lowering=False, debug=False)

    # all host layouts are partition-major: one clean DMA per tensor
    enc_nat_d = nc.declare_dram_parameter("enc_nat", [P, 8, H], BF16, isOutput=False)
    encT_d = nc.declare_dram_parameter("encT", [P, HC, T_ENC], F32, isOutput=False)
    decT_d = nc.declare_dram_parameter("decT", [P, HC, DPC], F32, isOutput=False)
    W_d = nc.declare_dram_parameter("W", [P, HC, HC, P], F32, isOutput=False)
    U_d = nc.declare_dram_parameter("U", [P, HC, HC, P], F32, isOutput=False)
    Vbig_d = nc.declare_dram_parameter("Vbig", [P, HC, 2 * P], BF16, isOutput=False)
    ident_d = nc.declare_dram_parameter("ident", [P, P], F32, isOutput=False)
    w_out_d = nc.declare_dram_parameter("w_out", [DPC, T_ENC], F32, isOutput=True)
    ctx_out_d = nc.declare_dram_parameter("ctx_out", [DPC, H], F32, isOutput=True)

    TANH = mybir.ActivationFunctionType.Tanh
    EXP = mybir.ActivationFunctionType.Exp

    with tile.TileContext(nc) as tc:
        with (
            tc.tile_pool(name="const", bufs=1) as cpool,
            tc.tile_pool(name="psum_big", bufs=1, space="PSUM") as pbig,
            tc.tile_pool(name="psum_aux", bufs=1, space="PSUM") as paux,
            tc.tile_pool(name="work", bufs=1) as wpool,
            tc.tile_pool(name="epi", bufs=1) as epool,
        ):
            # ---------------- constants in (one DMA each) ----------------
            encT_sb = cpool.tile([P, HC, T_ENC], F32, tag="encT_sb")
            nc.sync.dma_start(out=encT_sb[:], in_=encT_d[:])
            W_sb = cpool.tile([P, HC, HC, P], F32, tag="W_sb")
            nc.sync.dma_start(out=W_sb[:], in_=W_d[:])
            U_sb = cpool.tile([P, HC, HC, P], F32, tag="U_sb")
            nc.sync.dma_start(out=U_sb[:], in_=U_d[:])
            decT_sb = cpool.tile([P, HC, DPC], F32, tag="decT_sb")
            nc.sync.dma_start(out=decT_sb[:], in_=decT_d[:])
            Vbig_sb = cpool.tile([P, HC, 2 * P], BF16, tag="Vbig_sb")
            nc.sync.dma_start(out=Vbig_sb[:], in_=Vbig_d[:])
            ident_sb = cpool.tile([P, P], F32, tag="ident_sb")
            nc.sync.dma_start(out=ident_sb[:], in_=ident_d[:])
            enc_nat_sb = cpool.tile([P, 8, H], BF16, tag="enc_nat_sb")
            nc.sync.dma_start(out=enc_nat_sb[:], in_=enc_nat_d[:])

            # ---------------- projections ----------------
            encproj_ps = []
            for co in range(HC):
                ep = pbig.tile([P, T_ENC], F32, tag=f"encproj{co}", name=f"encproj{co}")
                encproj_ps.append(ep)
                for half in range(2):
                    sl = slice(half * 512, (half + 1) * 512)
                    for ci in range(HC):
                        nc.tensor.matmul(
                            ep[:, sl],
                            W_sb[:, ci, co, :],
                            encT_sb[:, ci, sl],
                            start=(ci == 0),
                            stop=(ci == HC - 1),
                        )
            decproj_ps = paux.tile([P, HC, DPC], F32, tag="aux", bufs=2, name="decproj_ps")
            for co in range(HC):
                for ci in range(HC):
                    nc.tensor.matmul(
                        decproj_ps[:, co, :],
                        U_sb[:, ci, co, :],
                        decT_sb[:, ci, :],
                        start=(ci == 0),
                        stop=(ci == HC - 1),
                    )
            decproj_sb = cpool.tile([P, HC, DPC], F32, tag="decproj_sb")
            nc.vector.tensor_copy(decproj_sb[:], decproj_ps[:])

            # bf16 copy of enc_projT in SBUF for the 4x DVE pre-add
            encproj_bf = cpool.tile([P, HC, T_ENC], BF16, tag="encproj_bf")
            for c in range(HC):
                nc.vector.tensor_copy(encproj_bf[:, c, :], encproj_ps[c][:])

            score_ps = pbig.tile([P, T_ENC], F32, tag="score", name="score")

            # ---------------- main loop: pre-add, tanh, V-reduction ------
            d0 = 0
            for g, G in enumerate(GROUPS):
                th = wpool.tile(
                    [P, GMAX, HC, T_ENC], BF16, tag="th", bufs=2, name=f"th{g}"
                )
                for r in range(G):
                    d = d0 + r
                    for c in range(HC):
                        nc.vector.tensor_scalar_add(
                            th[:, r, c, :],
                            encproj_bf[:, c, :],
                            decproj_sb[:, c, d : d + 1],
                        )
                # in-place tanh over the whole group (engines stream
                # read-before-write, so src == dst is safe)
                nc.scalar.activation(
                    th[:, :G, :, :], th[:, :G, :, :], TANH
                )
                for r in range(G):
                    d = d0 + r
                    for c in range(HC):
                        for half in range(2):
                            sl = slice(half * 512, (half + 1) * 512)
                            nc.tensor.matmul(
                                score_ps[:, sl],
                                Vbig_sb[:, c, P - d : 2 * P - d],
                                th[:, r, c, sl],
                                start=(d == 0 and c == 0),
                                stop=(d == DPC - 1 and c == HC - 1),
                            )
                d0 += G

            # ---------------- softmax (no max subtraction) ----------------
            expw = epool.tile([P, T_ENC], F32, tag="expw")
            sumexp = epool.tile([P, 1], F32, tag="sumexp")
            nc.scalar.activation(expw[:], score_ps[:], EXP, accum_out=sumexp[:])
            rec = epool.tile([P, 1], F32, tag="rec")
            nc.vector.reciprocal(rec[:], sumexp[:])
            wnorm = epool.tile([P, T_ENC], F32, tag="wnorm")
            nc.vector.tensor_scalar_mul(wnorm[:], expw[:], rec[:])
            nc.sync.dma_start(out=w_out_d[:], in_=wnorm[:])

            # ---------------- context = w @ enc (bf16 matmuls) ------------
            wT_sb = epool.tile([P, 8, DPC], BF16, tag="wT_sb")
            for t in range(8):
                wT_ps = paux.tile([P, P], F32, tag="aux", bufs=2, name=f"wT{t}")
                nc.tensor.transpose(
                    wT_ps[:], wnorm[:, t * P : (t + 1) * P], ident_sb[:]
                )
                nc.vector.tensor_copy(wT_sb[:, t, :], wT_ps[:])
            ctx_ps = paux.tile([P, H], F32, tag="aux", bufs=2, name="ctx_ps")
            for t in range(8):
                nc.tensor.matmul(
                    ctx_ps[:],
                    wT_sb[:, t, :],
                    enc_nat_sb[:, t, :],
                    start=(t == 0),
                    stop=(t == 7),
                )
            ctx_sb = epool.tile([P, H], F32, tag="ctx_sb")
            nc.vector.tensor_copy(ctx_sb[:], ctx_ps[:])
            nc.sync.dma_start(out=ctx_out_d[:], in_=ctx_sb[:])

    nc.compile()
    return nc


def make_in_maps(encoder_outputs, decoder_outputs, W_a, U_a, V_a):
    enc = np.ascontiguousarray(np.asarray(encoder_outputs, dtype=np.float32))
    dec = np.ascontiguousarray(np.asarray(decoder_outputs, dtype=np.float32))
    W = np.asarray(W_a, dtype=np.float32)
    U = np.asarray(U_a, dtype=np.float32)
    V = np.asarray(V_a, dtype=np.float32).reshape(H)

    # partition-major host layouts (single DMA per tensor)
    # enc_nat[p, t, h] = enc[b, t*128+p, h], in bf16
    enc_nat_all = np.ascontiguousarray(
        enc.reshape(B, 8, P, H).transpose(0, 2, 1, 3)
    ).astype(ml_dtypes.bfloat16)  # [b, p, t, h]
    # encT[p, c, e] = enc[b, e, c*128+p]
    encT_all = np.ascontiguousarray(
        enc.transpose(0, 2, 1).reshape(B, HC, P, T_ENC).transpose(0, 2, 1, 3)
    )  # [b, p, c, e]
    decT_full = dec.transpose(0, 2, 1).reshape(B, HC, P, T_DEC)  # [b, c, p, d]
    Wr = np.ascontiguousarray(
        W.reshape(HC, P, HC, P).transpose(1, 0, 2, 3)
    )  # [p, ci, co, n]
    Ur = np.ascontiguousarray(U.reshape(HC, P, HC, P).transpose(1, 0, 2, 3))

    Vbig = np.zeros((P, HC, 2 * P), dtype=ml_dtypes.bfloat16)
    for c in range(HC):
        Vbig[:, c, P] = V[c * P : (c + 1) * P].astype(ml_dtypes.bfloat16)
    ident = np.eye(P, dtype=np.float32)

    in_maps = []
    for core in range(N_CORES):
        b, half = core // 2, core % 2
        dlo = half * DPC
        decT_core = np.ascontiguousarray(
            decT_full[b][:, :, dlo : dlo + DPC].transpose(1, 0, 2)
        )  # [p, c, d]
        in_maps.append(
            {
                "enc_nat": enc_nat_all[b],
                "encT": encT_all[b],
                "decT": decT_core,
                "W": Wr,
                "U": Ur,
                "Vbig": Vbig,
                "ident": ident,
            }
        )
    return in_maps


def kernel(encoder_outputs, decoder_outputs, W_a, U_a, V_a):
    from concourse.bass_utils import run_bass_kernel_spmd

    in_maps = make_in_maps(encoder_outputs, decoder_outputs, W_a, U_a, V_a)
    nc = build_graph()
    res = run_bass_kernel_spmd(nc, in_maps, core_ids=list(range(N_CORES)))

    ctx = np.zeros((B, T_DEC, H), dtype=np.float32)
    w = np.zeros((B, T_DEC, T_ENC), dtype=np.float32)
    for core in range(N_CORES):
        b, half = core // 2, core % 2
        dlo = half * DPC
        out = res.results[core]
        ctx[b, dlo : dlo + DPC] = out["ctx_out"]
        w[b, dlo : dlo + DPC] = out["w_out"]
    return ctx, w


# revision 5
# speedup vs baseline: 1.0968x; 1.0968x over previous
"""Bahdanau additive attention on 8 TRN2 NeuronCores.

Problem shapes: encoder [4, 1024, 256], decoder [4, 256, 256],
W_a/U_a [256, 256], V_a [256, 1].
reference:
    enc_proj = enc @ W_a                  [B, E, H]
    dec_proj = dec @ U_a                  [B, D, H]
    score[b,d,e] = sum_h V[h] * tanh(dec_proj[b,d,h] + enc_proj[b,e,h])
    w = softmax(score, axis=-1)           [B, D, E]
    ctx = w @ enc                         [B, D, H]
    return (ctx, w)

Sharding: 8 cores = (batch b = core//2) x (decoder-row half = core%2).
Each core owns 128 decoder rows of one batch element; outputs are
disjoint so no collectives are needed.

Per-core dataflow (h on SBUF partitions, 2 chunks of 128):
  - TensorE: bf16 projections enc_projT[h,e], dec_projT[h,d]
    (contract over h_in on partitions; host ships transposed layouts).
  - VectorE: bf16 tensor_scalar pre-add (4x mode) builds
    arg[h, (r,c,e)] = enc_projT[h,e] + dec_projT[h,d].
  - ScalarE (the bottleneck, ~222us of 1-elem/cycle/lane tanh): ONE
    in-place tanh ACTIVATE per group of 8 decoder rows (FD=16384)
    amortizes the fixed ~224-cycle cost; groups stream back-to-back.
  - TensorE: score rows via the shifted-V trick: lhsT is a 128-col
    slice of a [128, 256] tensor whose only nonzero column (index 128)
    holds V; slice [128-d, 256-d) puts V in column d, so PSUM row d
    accumulates score[d, :] while other rows get += 0.  Interleaved
    FILLER matmuls (into a scratch bank) keep TensorE's HAM activity
    window busy so the real matmuls run at the warm 2.4 GHz rate and
    the last group's straggler tail stays small.
  - softmax along the free axis (exp + accum_out; max subtraction is
    skipped: |score| <= sum|V| ~ 10, far from fp32 overflow),
    TensorE transposes of w, bf16 context matmul, DMA out.
"""

import os
import sys

for _p in (
    "/opt/trn_rl_repo",
    "/root/.axon_site",
    "/root/.axon_site/_ro/trn_rl_repo",
    "/root/.axon_site/_ro/pypackages",
):
    if os.path.isdir(_p) and _p not in sys.path:
        sys.path.append(_p)

import ml_dtypes
import numpy as np

import concourse.mybir as mybir
from concourse import bacc, bass, tile

F32 = mybir.dt.float32
BF16 = mybir.dt.bfloat16

B, T_ENC, T_DEC, H = 4, 1024, 256, 256
P = 128  # SBUF partitions
HC = H // P  # h chunks (2)
DPC = 128  # decoder rows per core
N_CORES = 8

GROUPS = [4] + [8] * 15 + [4]
assert sum(GROUPS) == DPC
GMAX = max(GROUPS)
# filler matmuls per group: pad PE work to ~the ACT group duration
# ACT group ~= (224 + G*2048)/1.2 ns; warm matmul ~= 218 ns (N=512)
FILLERS = {4: 16, 8: 32}

MODE = os.environ.get("ATTN_KERNEL_MODE", "v4")


def build_graph(mode=MODE):
    nc = bacc.Bacc("TRN2", target_bir_lowering=False, debug=False)

    # partition-major host layouts: one clean DMA per tensor
    enc_nat_d = nc.declare_dram_parameter("enc_nat", [P, 8, H], BF16, isOutput=False)
    encT_d = nc.declare_dram_parameter("encT", [P, HC, T_ENC], BF16, isOutput=False)
    decT_d = nc.declare_dram_parameter("decT", [P, HC, DPC], BF16, isOutput=False)
    W_d = nc.declare_dram_parameter("W", [P, HC, HC, P], BF16, isOutput=False)
    U_d = nc.declare_dram_parameter("U", [P, HC, HC, P], BF16, isOutput=False)
    Vbig_d = nc.declare_dram_parameter("Vbig", [P, HC, 2 * P], BF16, isOutput=False)
    ident_d = nc.declare_dram_parameter("ident", [P, P], F32, isOutput=False)
    w_out_d = nc.declare_dram_parameter("w_out", [DPC, T_ENC], F32, isOutput=True)
    ctx_out_d = nc.declare_dram_parameter("ctx_out", [DPC, H], F32, isOutput=True)

    TANH = mybir.ActivationFunctionType.Tanh
    EXP = mybir.ActivationFunctionType.Exp

    with tile.TileContext(nc) as tc:
        with (
            tc.tile_pool(name="const", bufs=1) as cpool,
            tc.tile_pool(name="psum_big", bufs=1, space="PSUM") as pbig,
            tc.tile_pool(name="psum_aux", bufs=1, space="PSUM") as paux,
            tc.tile_pool(name="work", bufs=1) as wpool,
            tc.tile_pool(name="epi", bufs=1) as epool,
        ):
            # ---------------- constants in (one DMA each) ----------------
            encT_sb = cpool.tile([P, HC, T_ENC], BF16, tag="encT_sb")
            nc.sync.dma_start(out=encT_sb[:], in_=encT_d[:])
            W_sb = cpool.tile([P, HC, HC, P], BF16, tag="W_sb")
            nc.sync.dma_start(out=W_sb[:], in_=W_d[:])
            U_sb = cpool.tile([P, HC, HC, P], BF16, tag="U_sb")
            nc.sync.dma_start(out=U_sb[:], in_=U_d[:])
            decT_sb = cpool.tile([P, HC, DPC], BF16, tag="decT_sb")
            nc.sync.dma_start(out=decT_sb[:], in_=decT_d[:])
            Vbig_sb = cpool.tile([P, HC, 2 * P], BF16, tag="Vbig_sb")
            nc.sync.dma_start(out=Vbig_sb[:], in_=Vbig_d[:])
            ident_sb = cpool.tile([P, P], F32, tag="ident_sb")
            nc.sync.dma_start(out=ident_sb[:], in_=ident_d[:])
            enc_nat_sb = cpool.tile([P, 8, H], BF16, tag="enc_nat_sb")
            nc.sync.dma_start(out=enc_nat_sb[:], in_=enc_nat_d[:])

            # ---------------- projections (bf16 matmuls) ----------------
            encproj_ps = []
            for co in range(HC):
                ep = pbig.tile([P, T_ENC], F32, tag=f"encproj{co}", name=f"encproj{co}")
                encproj_ps.append(ep)
                for half in range(2):
                    sl = slice(half * 512, (half + 1) * 512)
                    for ci in range(HC):
                        nc.tensor.matmul(
                            ep[:, sl],
                            W_sb[:, ci, co, :],
                            encT_sb[:, ci, sl],
                            start=(ci == 0),
                            stop=(ci == HC - 1),
                        )
            decproj_ps = paux.tile(
                [P, HC, DPC], F32, tag="aux", bufs=2, name="decproj_ps"
            )
            for co in range(HC):
                for ci in range(HC):
                    nc.tensor.matmul(
                        decproj_ps[:, co, :],
                        U_sb[:, ci, co, :],
                        decT_sb[:, ci, :],
                        start=(ci == 0),
                        stop=(ci == HC - 1),
                    )
            decproj_sb = cpool.tile([P, HC, DPC], F32, tag="decproj_sb")
            nc.vector.tensor_copy(decproj_sb[:], decproj_ps[:])

            # bf16 copy of enc_projT in SBUF for the 4x DVE pre-add
            encproj_bf = cpool.tile([P, HC, T_ENC], BF16, tag="encproj_bf")
            for c in range(HC):
                nc.vector.tensor_copy(encproj_bf[:, c, :], encproj_ps[c][:])

            score_ps = pbig.tile([P, T_ENC], F32, tag="score", name="score")
            # scratch bank for HAM-warming filler matmuls: reuses the
            # encproj0 slot, which is dead once encproj_bf is built
            scratch_ps = pbig.tile([P, 512], F32, tag="encproj0", name="scratch")

            # ---------------- main loop ----------------
            d0 = 0
            for g, G in enumerate(GROUPS):
                th = wpool.tile(
                    [P, GMAX, HC, T_ENC], BF16, tag="th", bufs=3, name=f"th{g}"
                )
                for r in range(G):
                    d = d0 + r
                    for c in range(HC):
                        nc.vector.tensor_scalar_add(
                            th[:, r, c, :],
                            encproj_bf[:, c, :],
                            decproj_sb[:, c, d : d + 1],
                        )
                # in-place tanh over the whole group (engines stream
                # read-before-write, so src == dst is safe)
                nc.scalar.activation(th[:, :G, :, :], th[:, :G, :, :], TANH)
                n_fill = FILLERS[G] if g < len(GROUPS) - 2 else 0
                fill_per_r = (n_fill + G - 1) // G if n_fill else 0
                for r in range(G):
                    d = d0 + r
                    for c in range(HC):
                        for half in range(2):
                            sl = slice(half * 512, (half + 1) * 512)
                            nc.tensor.matmul(
                                score_ps[:, sl],
                                Vbig_sb[:, c, P - d : 2 * P - d],
                                th[:, r, c, sl],
                                start=(d == 0 and c == 0),
                                stop=(d == DPC - 1 and c == HC - 1),
                            )
                    for _ in range(fill_per_r):
                        nc.tensor.matmul(
                            scratch_ps[:],
                            Vbig_sb[:, 0, 0:P],
                            th[:, r, 0, 0:512],
                            start=True,
                            stop=True,
                            skip_group_check=True,
                        )
                d0 += G

            # ---------------- softmax (no max subtraction) ----------------
            expw = epool.tile([P, T_ENC], F32, tag="expw")
            sumexp = epool.tile([P, 1], F32, tag="sumexp")
            nc.scalar.activation(expw[:], score_ps[:], EXP, accum_out=sumexp[:])
            rec = epool.tile([P, 1], F32, tag="rec")
            nc.vector.reciprocal(rec[:], sumexp[:])
            wnorm = epool.tile([P, T_ENC], F32, tag="wnorm")
            nc.vector.tensor_scalar_mul(wnorm[:], expw[:], rec[:])
            nc.sync.dma_start(out=w_out_d[:], in_=wnorm[:])

            # ---------------- context = w @ enc (bf16 matmuls) ------------
            wT_sb = epool.tile([P, 8, DPC], BF16, tag="wT_sb")
            for t in range(8):
                wT_ps = paux.tile([P, P], F32, tag="aux", bufs=2, name=f"wT{t}")
                nc.tensor.transpose(
                    wT_ps[:], wnorm[:, t * P : (t + 1) * P], ident_sb[:]
                )
                nc.vector.tensor_copy(wT_sb[:, t, :], wT_ps[:])
            ctx_ps = paux.tile([P, H], F32, tag="aux", bufs=2, name="ctx_ps")
            for t in range(8):
                nc.tensor.matmul(
                    ctx_ps[:],
                    wT_sb[:, t, :],
                    enc_nat_sb[:, t, :],
                    start=(t == 0),
                    stop=(t == 7),
                )
            ctx_sb = epool.tile([P, H], F32, tag="ctx_sb")
            nc.vector.tensor_copy(ctx_sb[:], ctx_ps[:])
            nc.sync.dma_start(out=ctx_out_d[:], in_=ctx_sb[:])

    nc.compile()
    return nc


def make_in_maps(encoder_outputs, decoder_outputs, W_a, U_a, V_a):
    bf = ml_dtypes.bfloat16
    enc = np.ascontiguousarray(np.asarray(encoder_outputs, dtype=np.float32))
    dec = np.ascontiguousarray(np.asarray(decoder_outputs, dtype=np.float32))
    W = np.asarray(W_a, dtype=np.float32)
    U = np.asarray(U_a, dtype=np.float32)
    V = np.asarray(V_a, dtype=np.float32).reshape(H)

    # partition-major host layouts (single DMA per tensor)
    enc_nat_all = np.ascontiguousarray(
        enc.reshape(B, 8, P, H).transpose(0, 2, 1, 3)
    ).astype(bf)  # [b, p, t, h]
    encT_all = np.ascontiguousarray(
        enc.transpose(0, 2, 1).reshape(B, HC, P, T_ENC).transpose(0, 2, 1, 3)
    ).astype(bf)  # [b, p, c, e]
    decT_full = dec.transpose(0, 2, 1).reshape(B, HC, P, T_DEC)  # [b, c, p, d]
    Wr = np.ascontiguousarray(
        W.reshape(HC, P, HC, P).transpose(1, 0, 2, 3)
    ).astype(bf)  # [p, ci, co, n]
    Ur = np.ascontiguousarray(U.reshape(HC, P, HC, P).transpose(1, 0, 2, 3)).astype(bf)

    Vbig = np.zeros((P, HC, 2 * P), dtype=bf)
    for c in range(HC):
        Vbig[:, c, P] = V[c * P : (c + 1) * P].astype(bf)
    ident = np.eye(P, dtype=np.float32)

    in_maps = []
    for core in range(N_CORES):
        b, half = core // 2, core % 2
        dlo = half * DPC
        decT_core = np.ascontiguousarray(
            decT_full[b][:, :, dlo : dlo + DPC].transpose(1, 0, 2)
        ).astype(bf)  # [p, c, d]
        in_maps.append(
            {
                "enc_nat": enc_nat_all[b],
                "encT": encT_all[b],
                "decT": decT_core,
                "W": Wr,
                "U": Ur,
                "Vbig": Vbig,
                "ident": ident,
            }
        )
    return in_maps


def kernel(encoder_outputs, decoder_outputs, W_a, U_a, V_a):
    from concourse.bass_utils import run_bass_kernel_spmd

    in_maps = make_in_maps(encoder_outputs, decoder_outputs, W_a, U_a, V_a)
    nc = build_graph()
    res = run_bass_kernel_spmd(nc, in_maps, core_ids=list(range(N_CORES)))

    ctx = np.zeros((B, T_DEC, H), dtype=np.float32)
    w = np.zeros((B, T_DEC, T_ENC), dtype=np.float32)
    for core in range(N_CORES):
        b, half = core // 2, core % 2
        dlo = half * DPC
        out = res.results[core]
        ctx[b, dlo : dlo + DPC] = out["ctx_out"]
        w[b, dlo : dlo + DPC] = out["w_out"]
    return ctx, w
